# revision 22
# baseline (speedup 1.0000x reference)
"""Trainium2 Bass kernel for nn_CrossAttention (B=2, T=2048, D=1024, H=16, hd=64).

Sharding: 32 (batch, head) units over 8 cores -> each core handles 1 batch and
4 contiguous heads (core c: batch c//4, heads (c%4)*4 .. +4).  Per-core kernel
computes the partial c_proj output for its 4 heads; host sums the 4 partials
per batch and adds bc.

Per-core dataflow (all activations kept transposed, D-on-partitions):
  qpT [256, 2048] = WqT.T @ qT   (+bq)      kpT likewise
  per head h: S.T[tv, tq] = khT.T-slice matmuls (K=64)
              expS = exp(S.T / 8)           (ScalarE, scale fused)
              y_ext[65, 2048] += [vh|1].T @ expS   (ones row -> colsum)
              yT = y_ext[:64] * (1/colsum) (recip + DMA partition-broadcast)
  out_partial[2048, 1024] = yallT.T @ WcT  (K=256)

All matmuls run as float32r (full-rate fp32 PE mode).
"""

import sys

sys.path.insert(0, "/opt/trn_rl_repo")

import numpy as np

import concourse.bacc as bacc
import concourse.bass as bass
import concourse.mybir as mybir
import concourse.tile as tile
from concourse.bass_utils import run_bass_kernel_spmd

F32 = mybir.dt.float32
F32R = mybir.dt.float32r

T = 2048          # sequence length (both q and kv)
D = 1024          # model dim
HL = 4            # heads per core
HD = 64           # head dim
DH = HL * HD      # 256 local projected dim
P = 128
JT_G = DH // P  # 2
SCALE = 1.0 / 8.0  # 1/sqrt(64)

N_CORES = 8

_cache = {}


def r(ap):
    return ap.bitcast(F32R)


def build_nc():
    if "nc" in _cache:
        return _cache["nc"]
    nc = bacc.Bacc(
        "TRN2",
        target_bir_lowering=False,
        debug=False,
        num_devices=N_CORES,
    )

    qT = nc.declare_dram_parameter("qT", [D, T], F32R, isOutput=False)
    kT = nc.declare_dram_parameter("kT", [D, T], F32R, isOutput=False)
    v_sl = nc.declare_dram_parameter("v_sl", [T, DH], F32R, isOutput=False)
    WqT = nc.declare_dram_parameter("WqT", [D, DH], F32R, isOutput=False)
    WkT = nc.declare_dram_parameter("WkT", [D, DH], F32R, isOutput=False)
    WcT = nc.declare_dram_parameter("WcT", [DH, D], F32R, isOutput=False)
    bqk = nc.declare_dram_parameter("bqk", [P, 4], F32, isOutput=False)
    ones = nc.declare_dram_parameter("ones", [P, T // P], F32R, isOutput=False)
    out = nc.declare_dram_parameter("out", [T, D], F32, isOutput=True)
    debug = bool(int(__import__("os").environ.get("BASSDBG", "0")))
    if debug:
        dbg_qpT = nc.declare_dram_parameter("dbg_qpT", [P, JT_G, T], F32R, isOutput=True)
        dbg_kpT = nc.declare_dram_parameter("dbg_kpT", [P, JT_G, T], F32R, isOutput=True)
        dbg_es = nc.declare_dram_parameter("dbg_es", [P, 1024], F32R, isOutput=True)
        dbg_yall = nc.declare_dram_parameter("dbg_yall", [P, JT_G, T], F32R, isOutput=True)
        dbg_rec = nc.declare_dram_parameter("dbg_rec", [HD, T], F32, isOutput=True)
        dbg_col = nc.declare_dram_parameter("dbg_col", [1, T], F32, isOutput=True)
        dbg_ve = nc.declare_dram_parameter("dbg_ve", [P, T // P, HD + 1], F32R, isOutput=True)

    KT = D // P   # 8 din tiles
    JT = DH // P  # 2 dout tiles

    with tile.TileContext(nc) as tc:
        with (
            tc.tile_pool(name="wpool", bufs=1) as wpool,
            tc.tile_pool(name="stream", bufs=8) as stream,
            tc.tile_pool(name="projsb", bufs=1) as projsb,
            tc.tile_pool(name="vpool", bufs=1) as vpool,
            tc.tile_pool(name="epool", bufs=4) as epool,
            tc.tile_pool(name="npool", bufs=2) as npool,
            tc.tile_pool(name="opool", bufs=3) as opool,
            tc.tile_pool(name="psA", bufs=2, space="PSUM") as psA,
            tc.tile_pool(name="psB", bufs=1, space="PSUM") as psB,
            tc.tile_pool(name="drampool", bufs=2, space="DRAM") as drampool,
        ):
            # ---- weights / constants ----
            wq_sb = wpool.tile([P, KT, DH], F32R, name="wq_sb")
            nc.sync.dma_start(wq_sb[:], WqT.ap().rearrange("(a p) m -> p a m", p=P))
            wk_sb = wpool.tile([P, KT, DH], F32R, name="wk_sb")
            nc.sync.dma_start(wk_sb[:], WkT.ap().rearrange("(a p) m -> p a m", p=P))
            wc_sb = wpool.tile([P, JT, D], F32R, name="wc_sb")
            nc.sync.dma_start(wc_sb[:], WcT.ap().rearrange("(a p) m -> p a m", p=P))
            bias_sb = wpool.tile([P, 4], F32, name="bias_sb")  # [bq0,bq1,bk0,bk1]
            nc.sync.dma_start(bias_sb[:], bqk.ap())

            # ---- v_ext tiles: [v_h | ones] per head ----
            v_re = v_sl.ap().rearrange("(t p) d -> p t d", p=P)  # [128, 16, 256]
            vext = []
            for h in range(HL):
                ve = vpool.tile([P, T // P, HD + 1], F32R, name=f"vext{h}")
                nc.sync.dma_start(ve[:, :, 0:HD], v_re[:, :, h * HD:(h + 1) * HD])
                nc.sync.dma_start(
                    ve[:, :, HD:HD + 1],
                    ones.ap().unsqueeze(2),
                )
                vext.append(ve)

            # ---- projections: xpT[j*128+p, t] ----
            def project(xT_dram, w_sb, bias_col0, name):
                xpT = projsb.tile([P, JT, T], F32R, name=f"{name}pT")
                for cp in range(2):  # 1024-wide column groups of T
                    xt_tiles = []
                    for i in range(KT):
                        xt = stream.tile([P, 1024], F32R, tag="xt",
                                         name=f"{name}t{cp}_{i}")
                        nc.sync.dma_start(
                            xt[:],
                            xT_dram.ap()[i * P:(i + 1) * P,
                                         cp * 1024:(cp + 1) * 1024],
                        )
                        xt_tiles.append(xt)
                    groups = [
                        psA.tile([P, 1024], F32, tag="psA", name=f"{name}p{j}{cp}")
                        for j in range(JT)
                    ]
                    for i in range(KT):
                        for j in range(JT):
                            for c in range(2):
                                nc.tensor.matmul(
                                    groups[j][:, c * 512:(c + 1) * 512],
                                    w_sb[:, i, j * P:(j + 1) * P],
                                    xt_tiles[i][:, c * 512:(c + 1) * 512],
                                    start=(i == 0),
                                    stop=(i == KT - 1),
                                )
                    for j in range(JT):
                        nc.vector.tensor_tensor(
                            xpT[:, j, cp * 1024:(cp + 1) * 1024],
                            groups[j][:],
                            bias_sb[:, bias_col0 + j:bias_col0 + j + 1]
                            .to_broadcast((P, 1024)),
                            mybir.AluOpType.add,
                        )
                return xpT

            kpT = project(kT, wk_sb, 2, "k")
            qpT = project(qT, wq_sb, 0, "q")
            if debug:
                nc.sync.dma_start(dbg_qpT.ap(), qpT[:])
                nc.sync.dma_start(dbg_kpT.ap(), kpT[:])
                nc.sync.dma_start(dbg_ve.ap(), vext[0][:])

            # ---- attention per head ----
            yallT = projsb.tile([P, JT, T], F32R, name="yallT")
            for h in range(HL):
                j = h // 2
                p0 = (h % 2) * HD
                khT = kpT[p0:p0 + HD, j, :]   # [64, 2048]
                qhT = qpT[p0:p0 + HD, j, :]
                y_ps = psB.tile([HD + 1, T], F32, tag="psB", name=f"y{h}")
                for mv in range(T // P):
                    for half in range(2):
                        s_ps = psA.tile([P, 1024], F32, tag="psA", name=f"s{h}_{mv}_{half}")
                        for c in range(2):
                            q0 = half * 1024 + c * 512
                            nc.tensor.matmul(
                                s_ps[:, c * 512:(c + 1) * 512],
                                khT[:, mv * P:(mv + 1) * P],
                                qhT[:, q0:q0 + 512],
                                start=True,
                                stop=True,
                            )
                        es = epool.tile([P, 1024], F32R, tag="es", name=f"e{h}_{mv}_{half}")
                        nc.scalar.activation(
                            es[:], s_ps[:], mybir.ActivationFunctionType.Exp,
                            scale=SCALE,
                        )
                        if debug and h == 0 and mv == 0 and half == 0:
                            nc.sync.dma_start(dbg_es.ap(), es[:])
                        for c in range(2):
                            q0 = half * 1024 + c * 512
                            nc.tensor.matmul(
                                y_ps[:, q0:q0 + 512],
                                vext[h][:, mv, :],
                                es[:, c * 512:(c + 1) * 512],
                                start=(mv == 0),
                                stop=(mv == T // P - 1),
                            )
                # normalize: yT = y_ps[:64] * 1/colsum, place into yallT
                bcast = npool.tile([HD + 1, T], F32, tag="bcast", name=f"bc{h}")
                nc.vector.tensor_copy(bcast[HD:HD + 1, :], y_ps[HD:HD + 1, :])
                if debug and h == 0:
                    nc.sync.dma_start(dbg_col.ap(), bcast[HD:HD + 1, :])
                nc.vector.reciprocal(bcast[HD:HD + 1, :], bcast[HD:HD + 1, :])
                dscr = drampool.tile([1, T], F32, tag="dscr", name=f"dscr{h}")
                nc.sync.dma_start(dscr[:], bcast[HD:HD + 1, :])
                nc.sync.dma_start(bcast[0:HD, :], dscr[:].to_broadcast((HD, T)))
                ynorm = npool.tile([HD, T], F32R, tag="ynorm", name=f"yn{h}")
                nc.vector.tensor_tensor(
                    ynorm[:], y_ps[0:HD, :], bcast[0:HD, :], mybir.AluOpType.mult
                )
                nc.sync.dma_start(yallT[p0:p0 + HD, j, :], ynorm[:])
                if debug and h == 0:
                    nc.sync.dma_start(dbg_rec.ap(), bcast[0:HD, :])

            if debug:
                nc.sync.dma_start(dbg_yall.ap(), yallT[:])

            # ---- c_proj: out[t, :] = sum_j yallT[:,j,t].T @ wc[j] ----
            for mt in range(T // P):
                o_ps = psA.tile([P, 1024], F32, tag="psA", name=f"o{mt}")
                for nch in range(2):
                    for j in range(JT):
                        nc.tensor.matmul(
                            o_ps[:, nch * 512:(nch + 1) * 512],
                            yallT[:, j, mt * P:(mt + 1) * P],
                            wc_sb[:, j, nch * 512:(nch + 1) * 512],
                            start=(j == 0),
                            stop=(j == JT - 1),
                        )
                o_sb = opool.tile([P, 1024], F32, tag="osb", name=f"ot{mt}")
                nc.vector.tensor_copy(o_sb[:], o_ps[:])
                nc.sync.dma_start(out.ap()[mt * P:(mt + 1) * P, :], o_sb[:])

    nc.compile()
    _cache["nc"] = nc
    return nc


def make_in_maps(k, q, v, Wk, bk, Wq, bq, Wc, bc):
    k = np.asarray(k, dtype=np.float32)
    q = np.asarray(q, dtype=np.float32)
    v = np.asarray(v, dtype=np.float32)
    Wk = np.asarray(Wk, dtype=np.float32)
    Wq = np.asarray(Wq, dtype=np.float32)
    Wc = np.asarray(Wc, dtype=np.float32)
    bk = np.asarray(bk, dtype=np.float32)
    bq = np.asarray(bq, dtype=np.float32)
    in_maps = []
    for c in range(N_CORES):
        b = c // 4
        h0 = (c % 4) * HL
        sl = slice(h0 * HD, h0 * HD + DH)
        bq_t = np.ascontiguousarray(bq[sl].reshape(2, P).T)  # [128, 2]
        bk_t = np.ascontiguousarray(bk[sl].reshape(2, P).T)
        bqk = np.concatenate([bq_t, bk_t], axis=1)           # [128, 4]
        in_maps.append({
            "qT": np.ascontiguousarray(q[b].T),
            "kT": np.ascontiguousarray(k[b].T),
            "v_sl": np.ascontiguousarray(v[b][:, sl]),
            "WqT": np.ascontiguousarray(Wq[sl, :].T),
            "WkT": np.ascontiguousarray(Wk[sl, :].T),
            "WcT": np.ascontiguousarray(Wc[:, sl].T),
            "bqk": np.ascontiguousarray(bqk),
            "ones": np.ones((P, T // P), dtype=np.float32),
        })
    return in_maps


def kernel(k, q, v, Wk, bk, Wq, bq, Wc, bc, _trace=False, _trace_cores=None):
    bc = np.asarray(bc, dtype=np.float32)
    nc = build_nc()
    in_maps = make_in_maps(k, q, v, Wk, bk, Wq, bq, Wc, bc)
    res = run_bass_kernel_spmd(
        nc, in_maps, core_ids=list(range(N_CORES)),
        trace=_trace, trace_cores=_trace_cores,
    )
    outs = [res.results[c]["out"] for c in range(N_CORES)]
    full = np.stack([
        outs[0] + outs[1] + outs[2] + outs[3],
        outs[4] + outs[5] + outs[6] + outs[7],
    ]) + bc[None, None, :]
    kernel.last_result = res
    return full.astype(np.float32)
